# revision 19
# baseline (speedup 1.0000x reference)
"""GaussianMLP sampling kernel for 8 trn2 NeuronCores (pure data parallel).

reference:
    h      = relu(x @ W_emb + b_emb)        x:[B,128] W_emb:[128,256]
    mean   = h @ W_mean + b_mean            W_mean:[256,128]
    logvar = h @ W_logvar + b_logvar        W_logvar:[256,128]
    z      = mean + exp(0.5*logvar) * eps
    returns (z, mean, logvar)

Sharding: x/eps split along batch across 8 cores; weights replicated.

v3 design (memory-regime):
  - All bulk I/O in bf16 (host converts): halves HBM traffic. Outputs are
    packed into one [3, R, 128] DRAM tensor, written with ONE DMA per
    2048-row tile (4 KiB contiguous runs per partition).
  - DRAM views "(t p s) d -> t p (s d)" keep per-partition runs >= 4 KiB.
  - PE per 512-row chunk: 4 bf16 transposes (512 cyc) + L1 (1024 cyc) +
    L2 (2048 cyc, 8 matmuls of 256 cols into a combined [mean|logvar]
    PSUM tile). No bias matmuls: L1 bias rides the ACT relu; L2 biases
    are added by DVE/Pool from precomputed broadcast tiles.
  - 5-stage software pipeline so every engine runs dependency-free:
      A: transpose(c) [PE] + PSUM->SBUF copy [DVE]
      B: L1(c-1) [PE] + relu0/1(c-1) [ACT]
      C: L2(c-2) [PE]
      D: +b_mean(c-3) [DVE], +b_logvar(c-3) [Pool]
      E: exp(c-4) [ACT], se=std*eps(c-4) [DVE], z=mean+se(c-4) [DVE],
         output DMA (per tile) [Pool SWDGE queue]
"""

import sys

sys.path.insert(0, "/opt/trn_rl_repo")

import numpy as np
import ml_dtypes

from contextlib import ExitStack

from concourse import bacc, bass, masks, mybir, tile
from concourse.alu_op_type import AluOpType
from concourse.bass_utils import run_bass_kernel_spmd

N_CORES = 8
B = 524288
D_IN = 128
D_H = 256
D_OUT = 128
ROWS_PER_CORE = B // N_CORES  # 65536

S_DMA = 16  # rows-per-partition per input DMA tile (2048 rows)
CHUNK_S = 4  # 512-row compute chunk = 4 x 128-row subtiles
CHUNK = CHUNK_S * 128
CHUNKS_PER_TILE = S_DMA // CHUNK_S  # 4
TILE_ROWS = 128 * S_DMA  # 2048

F32 = mybir.dt.float32
BF16 = mybir.dt.bfloat16
NPBF16 = ml_dtypes.bfloat16


def build_bass(rows_per_core=ROWS_PER_CORE):
    nc = bacc.Bacc("TRN2", target_bir_lowering=False, debug=False)
    n_tiles = rows_per_core // TILE_ROWS
    n_chunks = rows_per_core // CHUNK

    x_ext = nc.declare_dram_parameter("x", [rows_per_core, D_IN], BF16, isOutput=False)
    eps_ext = nc.declare_dram_parameter(
        "eps", [rows_per_core, D_OUT], BF16, isOutput=False
    )
    We_ext = nc.declare_dram_parameter("W_emb", [D_IN, D_H], F32, isOutput=False)
    be_ext = nc.declare_dram_parameter("b_emb", [D_H], F32, isOutput=False)
    Wm_ext = nc.declare_dram_parameter("W_mean", [D_H, D_OUT], F32, isOutput=False)
    bm_ext = nc.declare_dram_parameter("b_mean", [D_OUT], F32, isOutput=False)
    Wl_ext = nc.declare_dram_parameter("W_logvar", [D_H, D_OUT], F32, isOutput=False)
    bl_ext = nc.declare_dram_parameter("b_logvar", [D_OUT], F32, isOutput=False)
    out_ext = nc.declare_dram_parameter(
        "out", [3, rows_per_core, D_OUT], BF16, isOutput=True
    )

    # row = t*TILE_ROWS + p*S_DMA + s ; per-partition contiguous run = s*d
    xv = x_ext.rearrange("(t p s) d -> t p (s d)", p=128, s=S_DMA)
    ev = eps_ext.rearrange("(t p s) d -> t p (s d)", p=128, s=S_DMA)
    ov = out_ext.rearrange("c (t p s) d -> t p c s d", p=128, s=S_DMA)

    with tile.TileContext(nc) as tc, ExitStack() as ctx:
        const = ctx.enter_context(tc.tile_pool(name="const", bufs=1))
        xin = ctx.enter_context(tc.tile_pool(name="xin", bufs=3))
        epool = ctx.enter_context(tc.tile_pool(name="eps", bufs=4))
        xTp = ctx.enter_context(tc.tile_pool(name="xT", bufs=3))
        hTp = ctx.enter_context(tc.tile_pool(name="hTs", bufs=3))
        stdp = ctx.enter_context(tc.tile_pool(name="std", bufs=2))
        sep = ctx.enter_context(tc.tile_pool(name="se", bufs=2))
        outs = ctx.enter_context(tc.tile_pool(name="outs", bufs=2))
        psA = ctx.enter_context(tc.tile_pool(name="psA", bufs=2, space="PSUM"))
        psB0 = ctx.enter_context(tc.tile_pool(name="psB0", bufs=1, space="PSUM"))
        psB1 = ctx.enter_context(tc.tile_pool(name="psB1", bufs=1, space="PSUM"))
        psC = ctx.enter_context(tc.tile_pool(name="psC", bufs=2, space="PSUM"))

        # --- constants / weights (loaded once) ---
        ident = const.tile([128, 128], BF16)
        masks.make_identity(nc, ident[:])

        We_sb = const.tile([128, D_H], BF16)
        nc.gpsimd.dma_start(We_sb[:], We_ext[:])
        # combined [W_mean | W_logvar]: [k-chunk partition, k, 2*D_OUT]
        Wml_sb = const.tile([128, 2, 2 * D_OUT], BF16)
        nc.gpsimd.dma_start(
            Wml_sb[:, :, 0:D_OUT], Wm_ext.rearrange("(c p) d -> p c d", p=128)
        )
        nc.gpsimd.dma_start(
            Wml_sb[:, :, D_OUT : 2 * D_OUT],
            Wl_ext.rearrange("(c p) d -> p c d", p=128),
        )

        be_sb = const.tile([128, 2], F32)
        nc.sync.dma_start(be_sb[:], be_ext.rearrange("(c p) -> p c", p=128))

        # broadcast b_mean/b_logvar across partitions via one-time K=1
        # matmuls: [128,CHUNK] = ones[1,128].T @ bias_rep[1,CHUNK]
        ones_sb = const.tile([1, 128], F32)
        nc.vector.memset(ones_sb[:], 1.0)
        bm_rep = const.tile([1, CHUNK], F32)
        bl_rep = const.tile([1, CHUNK], F32)
        for s in range(CHUNK_S):
            nc.sync.dma_start(
                bm_rep[0:1, s * D_OUT : (s + 1) * D_OUT],
                bm_ext.rearrange("(o d) -> o d", o=1),
            )
            nc.sync.dma_start(
                bl_rep[0:1, s * D_OUT : (s + 1) * D_OUT],
                bl_ext.rearrange("(o d) -> o d", o=1),
            )
        bm_bc = const.tile([128, CHUNK_S, D_OUT], F32)
        bl_bc = const.tile([128, CHUNK_S, D_OUT], F32)
        binit_ps = psC.tile([128, CHUNK_S, 2 * D_OUT], F32, tag="ml")
        nc.tensor.matmul(
            binit_ps[:].rearrange("p s d -> p (s d)")[:, 0:CHUNK],
            ones_sb[:],
            bm_rep[:],
            start=True,
            stop=True,
            skip_group_check=True,
        )
        nc.vector.tensor_copy(
            bm_bc[:].rearrange("p s d -> p (s d)"),
            binit_ps[:].rearrange("p s d -> p (s d)")[:, 0:CHUNK],
        )
        binit2_ps = psC.tile([128, CHUNK_S, 2 * D_OUT], F32, tag="ml")
        nc.tensor.matmul(
            binit2_ps[:].rearrange("p s d -> p (s d)")[:, 0:CHUNK],
            ones_sb[:],
            bl_rep[:],
            start=True,
            stop=True,
            skip_group_check=True,
        )
        nc.vector.tensor_copy(
            bl_bc[:].rearrange("p s d -> p (s d)"),
            binit2_ps[:].rearrange("p s d -> p (s d)")[:, 0:CHUNK],
        )

        # --- pipelined main loop ---
        x_tiles = {}
        eps_tiles = {}
        xT_sbs = {}
        hT_sbs = {}
        ml_pss = {}
        out_sbs = {}
        std_sbs = {}

        def fetch_x(t):
            if t >= n_tiles:
                return
            x_sb = xin.tile([128, S_DMA * D_IN], BF16, tag="x")
            nc.sync.dma_start(x_sb[:], xv[t])
            x_tiles[t] = x_sb

        def fetch_eps(t):
            if t >= n_tiles:
                return
            e_sb = epool.tile([128, S_DMA * D_OUT], BF16, tag="eps")
            nc.sync.dma_start(e_sb[:], ev[t])
            eps_tiles[t] = e_sb

        for t0 in (0, 1):
            fetch_x(t0)
            fetch_eps(t0)

        for c in range(n_chunks + 5):
            # ---- stage E part 1: exp for chunk c-4 (inputs ready since
            # last iteration -> keeps ACT dense from the iteration start) ----
            g = c - 4
            if 0 <= g < n_chunks:
                t_g, j_g = divmod(g, CHUNKS_PER_TILE)
                out_sb_g = out_sbs[t_g]
                ssl_g = slice(j_g * CHUNK_S, (j_g + 1) * CHUNK_S)
                std_sb = stdp.tile([128, CHUNK_S, D_OUT], BF16, tag="std")
                nc.scalar.activation(
                    std_sb[:],
                    out_sb_g[:, 2, ssl_g, :],
                    mybir.ActivationFunctionType.Exp,
                    scale=0.5,
                )
                std_sbs[g] = std_sb

            # ---- stage D: bias adds for chunk c-3 (L2 done last iter,
            # so these are ready first on DVE) ----
            f = c - 3
            if 0 <= f < n_chunks:
                t_f, j_f = divmod(f, CHUNKS_PER_TILE)
                if j_f == 0:
                    out_sbs[t_f] = outs.tile(
                        [128, 3, S_DMA, D_OUT], BF16, tag="o", name="out_sb"
                    )
                out_sb = out_sbs[t_f]
                ml_ps = ml_pss.pop(f)
                ssl = slice(j_f * CHUNK_S, (j_f + 1) * CHUNK_S)
                nc.vector.tensor_add(
                    out_sb[:, 1, ssl, :], ml_ps[:, :, 0:D_OUT], bm_bc[:]
                )
                nc.vector.tensor_add(
                    out_sb[:, 2, ssl, :], ml_ps[:, :, D_OUT : 2 * D_OUT], bl_bc[:]
                )

            # ---- stage A: transpose chunk c ----
            if c < n_chunks:
                t, j = divmod(c, CHUNKS_PER_TILE)
                if j == 0:
                    fetch_x(t + 2)
                elif j == 2:
                    fetch_eps(t + 2)
                x_sb = x_tiles[t]
                xT_ps = psA.tile([128, CHUNK], BF16, tag="xT")
                for q in range(CHUNK_S):
                    s = j * CHUNK_S + q
                    nc.tensor.transpose(
                        xT_ps[:, q * 128 : (q + 1) * 128],
                        x_sb[:, s * D_IN : (s + 1) * D_IN],
                        ident[:],
                    )
                xT_sb = xTp.tile([128, CHUNK], BF16, tag="xTs")
                nc.vector.tensor_copy(xT_sb[:], xT_ps[:])
                xT_sbs[c] = xT_sb
                if j == CHUNKS_PER_TILE - 1:
                    del x_tiles[t]

            # ---- stage B: L1 + relu for chunk c-1 ----
            d = c - 1
            if 0 <= d < n_chunks:
                xT_sb = xT_sbs.pop(d)
                hT_ps0 = psB0.tile([128, CHUNK], F32, tag="hT0")
                hT_ps1 = psB1.tile([128, CHUNK], F32, tag="hT1")
                nc.tensor.matmul(
                    hT_ps0[:], We_sb[:, 0:128], xT_sb[:], start=True, stop=True
                )
                nc.tensor.matmul(
                    hT_ps1[:], We_sb[:, 128:256], xT_sb[:], start=True, stop=True
                )
                hT_sb0 = hTp.tile([128, CHUNK], BF16, tag="h0")
                hT_sb1 = hTp.tile([128, CHUNK], BF16, tag="h1")
                nc.scalar.activation(
                    hT_sb0[:],
                    hT_ps0[:],
                    mybir.ActivationFunctionType.Relu,
                    bias=be_sb[:, 0:1],
                )
                nc.scalar.activation(
                    hT_sb1[:],
                    hT_ps1[:],
                    mybir.ActivationFunctionType.Relu,
                    bias=be_sb[:, 1:2],
                )
                hT_sbs[d] = (hT_sb0, hT_sb1)

            # ---- stage C: L2 for chunk c-2 ----
            e = c - 2
            if 0 <= e < n_chunks:
                hT_sb0, hT_sb1 = hT_sbs.pop(e)
                ml_ps = psC.tile([128, CHUNK_S, 2 * D_OUT], F32, tag="ml")
                for s in range(CHUNK_S):
                    sl = slice(s * 128, (s + 1) * 128)
                    for k, hT_sbk in ((0, hT_sb0), (1, hT_sb1)):
                        nc.tensor.matmul(
                            ml_ps[:, s, :],
                            hT_sbk[:, sl],
                            Wml_sb[:, k, :],
                            start=(k == 0),
                            stop=(k == 1),
                        )
                ml_pss[e] = ml_ps

            # ---- stage E part 2: sample / store for chunk c-4 ----
            if 0 <= g < n_chunks:
                t_g, j_g = divmod(g, CHUNKS_PER_TILE)
                out_sb = out_sbs[t_g]
                ssl = slice(j_g * CHUNK_S, (j_g + 1) * CHUNK_S)
                std_sb = std_sbs.pop(g)
                e_sb = eps_tiles[t_g]
                se_sb = sep.tile([128, CHUNK_S, D_OUT], BF16, tag="se")
                nc.vector.tensor_mul(
                    se_sb[:],
                    std_sb[:],
                    e_sb[:, j_g * CHUNK : (j_g + 1) * CHUNK].rearrange(
                        "p (s d) -> p s d", s=CHUNK_S
                    ),
                )
                nc.gpsimd.tensor_add(
                    out_sb[:, 0, ssl, :], out_sb[:, 1, ssl, :], se_sb[:]
                )
                half = S_DMA // 2
                if j_g == CHUNKS_PER_TILE // 2 - 1:
                    nc.gpsimd.dma_start(
                        ov[t_g][:, :, 0:half, :], out_sb[:, :, 0:half, :]
                    )
                elif j_g == CHUNKS_PER_TILE - 1:
                    nc.gpsimd.dma_start(
                        ov[t_g][:, :, half:S_DMA, :], out_sb[:, :, half:S_DMA, :]
                    )
                    del out_sbs[t_g]
                    del eps_tiles[t_g]

    nc.finalize()
    return nc


_NC_CACHE = None


def _get_nc():
    global _NC_CACHE
    if _NC_CACHE is None:
        _NC_CACHE = build_bass()
    return _NC_CACHE


def _run(inputs, trace=False, **kw):
    nc = _get_nc()
    xs = np.ascontiguousarray(np.asarray(inputs["x"])).astype(NPBF16)
    es = np.ascontiguousarray(np.asarray(inputs["eps"])).astype(NPBF16)
    weights = {
        k: np.ascontiguousarray(np.asarray(inputs[k], dtype=np.float32))
        for k in ("W_emb", "b_emb", "W_mean", "b_mean", "W_logvar", "b_logvar")
    }
    in_maps = []
    for c in range(N_CORES):
        sl = slice(c * ROWS_PER_CORE, (c + 1) * ROWS_PER_CORE)
        in_maps.append({"x": xs[sl], "eps": es[sl], **weights})
    res = run_bass_kernel_spmd(nc, in_maps, list(range(N_CORES)), trace=trace, **kw)
    z = np.concatenate(
        [res.results[c]["out"][0] for c in range(N_CORES)], axis=0
    ).astype(np.float32)
    mean = np.concatenate(
        [res.results[c]["out"][1] for c in range(N_CORES)], axis=0
    ).astype(np.float32)
    lv = np.concatenate(
        [res.results[c]["out"][2] for c in range(N_CORES)], axis=0
    ).astype(np.float32)
    return (z, mean, lv), res


def kernel(**inputs):
    out, _ = _run(inputs, trace=False)
    return out


if __name__ == "__main__":
    rng = np.random.default_rng(0)
    demo = {
        "x": rng.standard_normal((B, D_IN), dtype=np.float32),
        "eps": rng.standard_normal((B, D_OUT), dtype=np.float32),
        "W_emb": rng.standard_normal((D_IN, D_H), dtype=np.float32) * 0.088,
        "b_emb": rng.standard_normal((D_H,), dtype=np.float32) * 0.05,
        "W_mean": rng.standard_normal((D_H, D_OUT), dtype=np.float32) * 0.06,
        "b_mean": rng.standard_normal((D_OUT,), dtype=np.float32) * 0.03,
        "W_logvar": rng.standard_normal((D_H, D_OUT), dtype=np.float32) * 0.06,
        "b_logvar": rng.standard_normal((D_OUT,), dtype=np.float32) * 0.03,
    }
    z, m, l = kernel(**demo)
    print("shapes", z.shape, m.shape, l.shape)


# revision 20
# speedup vs baseline: 1.0445x; 1.0445x over previous
"""GaussianMLP sampling kernel for 8 trn2 NeuronCores (pure data parallel).

reference:
    h      = relu(x @ W_emb + b_emb)        x:[B,128] W_emb:[128,256]
    mean   = h @ W_mean + b_mean            W_mean:[256,128]
    logvar = h @ W_logvar + b_logvar        W_logvar:[256,128]
    z      = mean + exp(0.5*logvar) * eps
    returns (z, mean, logvar)

Sharding: x/eps split along batch across 8 cores; weights replicated.

v3 design (memory-regime):
  - All bulk I/O in bf16 (host converts): halves HBM traffic. Outputs are
    packed into one [3, R, 128] DRAM tensor, written with ONE DMA per
    2048-row tile (4 KiB contiguous runs per partition).
  - DRAM views "(t p s) d -> t p (s d)" keep per-partition runs >= 4 KiB.
  - PE per 512-row chunk: 4 bf16 transposes (512 cyc) + L1 (1024 cyc) +
    L2 (2048 cyc, 8 matmuls of 256 cols into a combined [mean|logvar]
    PSUM tile). No bias matmuls: L1 bias rides the ACT relu; L2 biases
    are added by DVE/Pool from precomputed broadcast tiles.
  - 5-stage software pipeline so every engine runs dependency-free:
      A: transpose(c) [PE] + PSUM->SBUF copy [DVE]
      B: L1(c-1) [PE] + relu0/1(c-1) [ACT]
      C: L2(c-2) [PE]
      D: +b_mean(c-3) [DVE], +b_logvar(c-3) [Pool]
      E: exp(c-4) [ACT], se=std*eps(c-4) [DVE], z=mean+se(c-4) [DVE],
         output DMA (per tile) [Pool SWDGE queue]
"""

import sys

sys.path.insert(0, "/opt/trn_rl_repo")

import numpy as np
import ml_dtypes

from contextlib import ExitStack

from concourse import bacc, bass, masks, mybir, tile
from concourse.alu_op_type import AluOpType
from concourse.bass_utils import run_bass_kernel_spmd

N_CORES = 8
B = 524288
D_IN = 128
D_H = 256
D_OUT = 128
ROWS_PER_CORE = B // N_CORES  # 65536

S_DMA = 16  # rows-per-partition per input DMA tile (2048 rows)
CHUNK_S = 4  # 512-row compute chunk = 4 x 128-row subtiles
CHUNK = CHUNK_S * 128
CHUNKS_PER_TILE = S_DMA // CHUNK_S  # 4
TILE_ROWS = 128 * S_DMA  # 2048

F32 = mybir.dt.float32
BF16 = mybir.dt.bfloat16
NPBF16 = ml_dtypes.bfloat16


def build_bass(rows_per_core=ROWS_PER_CORE):
    nc = bacc.Bacc("TRN2", target_bir_lowering=False, debug=False)
    n_tiles = rows_per_core // TILE_ROWS
    n_chunks = rows_per_core // CHUNK

    x_ext = nc.declare_dram_parameter("x", [rows_per_core, D_IN], BF16, isOutput=False)
    eps_ext = nc.declare_dram_parameter(
        "eps", [rows_per_core, D_OUT], BF16, isOutput=False
    )
    We_ext = nc.declare_dram_parameter("W_emb", [D_IN, D_H], F32, isOutput=False)
    be_ext = nc.declare_dram_parameter("b_emb", [D_H], F32, isOutput=False)
    Wm_ext = nc.declare_dram_parameter("W_mean", [D_H, D_OUT], F32, isOutput=False)
    bm_ext = nc.declare_dram_parameter("b_mean", [D_OUT], F32, isOutput=False)
    Wl_ext = nc.declare_dram_parameter("W_logvar", [D_H, D_OUT], F32, isOutput=False)
    bl_ext = nc.declare_dram_parameter("b_logvar", [D_OUT], F32, isOutput=False)
    out_ext = nc.declare_dram_parameter(
        "out", [3, rows_per_core, D_OUT], BF16, isOutput=True
    )

    # row = t*TILE_ROWS + p*S_DMA + s ; per-partition contiguous run = s*d
    xv = x_ext.rearrange("(t p s) d -> t p (s d)", p=128, s=S_DMA)
    ev = eps_ext.rearrange("(t p s) d -> t p (s d)", p=128, s=S_DMA)
    ov = out_ext.rearrange("c (t p s) d -> t p c s d", p=128, s=S_DMA)

    with tile.TileContext(nc) as tc, ExitStack() as ctx:
        const = ctx.enter_context(tc.tile_pool(name="const", bufs=1))
        xin = ctx.enter_context(tc.tile_pool(name="xin", bufs=3))
        epool = ctx.enter_context(tc.tile_pool(name="eps", bufs=4))
        xTp = ctx.enter_context(tc.tile_pool(name="xT", bufs=3))
        hTp = ctx.enter_context(tc.tile_pool(name="hTs", bufs=3))
        stdp = ctx.enter_context(tc.tile_pool(name="std", bufs=2))
        sep = ctx.enter_context(tc.tile_pool(name="se", bufs=2))
        outs = ctx.enter_context(tc.tile_pool(name="outs", bufs=2))
        psA = ctx.enter_context(tc.tile_pool(name="psA", bufs=2, space="PSUM"))
        psB0 = ctx.enter_context(tc.tile_pool(name="psB0", bufs=1, space="PSUM"))
        psB1 = ctx.enter_context(tc.tile_pool(name="psB1", bufs=1, space="PSUM"))
        psC = ctx.enter_context(tc.tile_pool(name="psC", bufs=2, space="PSUM"))

        # --- constants / weights (loaded once) ---
        ident = const.tile([128, 128], BF16)
        masks.make_identity(nc, ident[:])

        We_sb = const.tile([128, D_H], BF16)
        nc.gpsimd.dma_start(We_sb[:], We_ext[:])
        # combined [W_mean | W_logvar]: [k-chunk partition, k, 2*D_OUT]
        Wml_sb = const.tile([128, 2, 2 * D_OUT], BF16)
        nc.gpsimd.dma_start(
            Wml_sb[:, :, 0:D_OUT], Wm_ext.rearrange("(c p) d -> p c d", p=128)
        )
        nc.gpsimd.dma_start(
            Wml_sb[:, :, D_OUT : 2 * D_OUT],
            Wl_ext.rearrange("(c p) d -> p c d", p=128),
        )

        be_sb = const.tile([128, 2], F32)
        nc.sync.dma_start(be_sb[:], be_ext.rearrange("(c p) -> p c", p=128))

        # broadcast b_mean/b_logvar across partitions via one-time K=1
        # matmuls: [128,CHUNK] = ones[1,128].T @ bias_rep[1,CHUNK]
        ones_sb = const.tile([1, 128], F32)
        nc.vector.memset(ones_sb[:], 1.0)
        bm_rep = const.tile([1, CHUNK], F32)
        bl_rep = const.tile([1, CHUNK], F32)
        for s in range(CHUNK_S):
            nc.sync.dma_start(
                bm_rep[0:1, s * D_OUT : (s + 1) * D_OUT],
                bm_ext.rearrange("(o d) -> o d", o=1),
            )
            nc.sync.dma_start(
                bl_rep[0:1, s * D_OUT : (s + 1) * D_OUT],
                bl_ext.rearrange("(o d) -> o d", o=1),
            )
        bm_bc = const.tile([128, CHUNK_S, D_OUT], F32)
        bl_bc = const.tile([128, CHUNK_S, D_OUT], F32)
        binit_ps = psC.tile([128, CHUNK_S, 2 * D_OUT], F32, tag="ml")
        nc.tensor.matmul(
            binit_ps[:].rearrange("p s d -> p (s d)")[:, 0:CHUNK],
            ones_sb[:],
            bm_rep[:],
            start=True,
            stop=True,
            skip_group_check=True,
        )
        nc.vector.tensor_copy(
            bm_bc[:].rearrange("p s d -> p (s d)"),
            binit_ps[:].rearrange("p s d -> p (s d)")[:, 0:CHUNK],
        )
        binit2_ps = psC.tile([128, CHUNK_S, 2 * D_OUT], F32, tag="ml")
        nc.tensor.matmul(
            binit2_ps[:].rearrange("p s d -> p (s d)")[:, 0:CHUNK],
            ones_sb[:],
            bl_rep[:],
            start=True,
            stop=True,
            skip_group_check=True,
        )
        nc.vector.tensor_copy(
            bl_bc[:].rearrange("p s d -> p (s d)"),
            binit2_ps[:].rearrange("p s d -> p (s d)")[:, 0:CHUNK],
        )

        # --- pipelined main loop ---
        x_tiles = {}
        eps_tiles = {}
        xT_sbs = {}
        hT_sbs = {}
        ml_pss = {}
        out_sbs = {}
        std_sbs = {}

        def fetch_x(t):
            if t >= n_tiles:
                return
            x_sb = xin.tile([128, S_DMA * D_IN], BF16, tag="x")
            nc.sync.dma_start(x_sb[:], xv[t])
            x_tiles[t] = x_sb

        def fetch_eps(t):
            if t >= n_tiles:
                return
            e_sb = epool.tile([128, S_DMA * D_OUT], BF16, tag="eps")
            nc.sync.dma_start(e_sb[:], ev[t])
            eps_tiles[t] = e_sb

        for t0 in (0, 1):
            fetch_x(t0)
            fetch_eps(t0)

        for c in range(n_chunks + 5):
            # ---- stage E part 1: exp for chunk c-4 (inputs ready since
            # last iteration -> keeps ACT dense from the iteration start) ----
            g = c - 4
            if 0 <= g < n_chunks:
                t_g, j_g = divmod(g, CHUNKS_PER_TILE)
                out_sb_g = out_sbs[t_g]
                ssl_g = slice(j_g * CHUNK_S, (j_g + 1) * CHUNK_S)
                std_sb = stdp.tile([128, CHUNK_S, D_OUT], BF16, tag="std")
                nc.scalar.activation(
                    std_sb[:],
                    out_sb_g[:, 2, ssl_g, :],
                    mybir.ActivationFunctionType.Exp,
                    scale=0.5,
                )
                std_sbs[g] = std_sb

            # ---- stage D: bias adds for chunk c-3 (L2 done last iter,
            # so these are ready first on DVE) ----
            f = c - 3
            if 0 <= f < n_chunks:
                t_f, j_f = divmod(f, CHUNKS_PER_TILE)
                if j_f == 0:
                    out_sbs[t_f] = outs.tile(
                        [128, 3, S_DMA, D_OUT], BF16, tag="o", name="out_sb"
                    )
                out_sb = out_sbs[t_f]
                ml_ps = ml_pss.pop(f)
                ssl = slice(j_f * CHUNK_S, (j_f + 1) * CHUNK_S)
                nc.vector.tensor_add(
                    out_sb[:, 1, ssl, :], ml_ps[:, :, 0:D_OUT], bm_bc[:]
                )
                nc.vector.tensor_add(
                    out_sb[:, 2, ssl, :], ml_ps[:, :, D_OUT : 2 * D_OUT], bl_bc[:]
                )

            # ---- stage A: transpose chunk c ----
            if c < n_chunks:
                t, j = divmod(c, CHUNKS_PER_TILE)
                if j == 0:
                    fetch_x(t + 2)
                elif j == 2:
                    fetch_eps(t + 2)
                x_sb = x_tiles[t]
                xT_ps = psA.tile([128, CHUNK], BF16, tag="xT")
                for q in range(CHUNK_S):
                    s = j * CHUNK_S + q
                    nc.tensor.transpose(
                        xT_ps[:, q * 128 : (q + 1) * 128],
                        x_sb[:, s * D_IN : (s + 1) * D_IN],
                        ident[:],
                    )
                xT_sb = xTp.tile([128, CHUNK], BF16, tag="xTs")
                nc.vector.tensor_copy(xT_sb[:], xT_ps[:])
                xT_sbs[c] = xT_sb
                if j == CHUNKS_PER_TILE - 1:
                    del x_tiles[t]

            # ---- stage B: L1 + relu for chunk c-1 ----
            d = c - 1
            if 0 <= d < n_chunks:
                xT_sb = xT_sbs.pop(d)
                hT_ps0 = psB0.tile([128, CHUNK], F32, tag="hT0")
                hT_ps1 = psB1.tile([128, CHUNK], F32, tag="hT1")
                nc.tensor.matmul(
                    hT_ps0[:], We_sb[:, 0:128], xT_sb[:], start=True, stop=True
                )
                nc.tensor.matmul(
                    hT_ps1[:], We_sb[:, 128:256], xT_sb[:], start=True, stop=True
                )
                hT_sb0 = hTp.tile([128, CHUNK], BF16, tag="h0")
                hT_sb1 = hTp.tile([128, CHUNK], BF16, tag="h1")
                nc.scalar.activation(
                    hT_sb0[:],
                    hT_ps0[:],
                    mybir.ActivationFunctionType.Relu,
                    bias=be_sb[:, 0:1],
                )
                nc.scalar.activation(
                    hT_sb1[:],
                    hT_ps1[:],
                    mybir.ActivationFunctionType.Relu,
                    bias=be_sb[:, 1:2],
                )
                hT_sbs[d] = (hT_sb0, hT_sb1)

            # ---- stage C: L2 for chunk c-2 ----
            e = c - 2
            if 0 <= e < n_chunks:
                hT_sb0, hT_sb1 = hT_sbs.pop(e)
                ml_ps = psC.tile([128, CHUNK_S, 2 * D_OUT], F32, tag="ml")
                for s in range(CHUNK_S):
                    sl = slice(s * 128, (s + 1) * 128)
                    for k, hT_sbk in ((0, hT_sb0), (1, hT_sb1)):
                        nc.tensor.matmul(
                            ml_ps[:, s, :],
                            hT_sbk[:, sl],
                            Wml_sb[:, k, :],
                            start=(k == 0),
                            stop=(k == 1),
                        )
                ml_pss[e] = ml_ps

            # ---- stage E part 2: sample / store for chunk c-4 ----
            if 0 <= g < n_chunks:
                t_g, j_g = divmod(g, CHUNKS_PER_TILE)
                out_sb = out_sbs[t_g]
                ssl = slice(j_g * CHUNK_S, (j_g + 1) * CHUNK_S)
                std_sb = std_sbs.pop(g)
                e_sb = eps_tiles[t_g]
                se_sb = sep.tile([128, CHUNK_S, D_OUT], BF16, tag="se")
                nc.vector.tensor_mul(
                    se_sb[:],
                    std_sb[:],
                    e_sb[:, j_g * CHUNK : (j_g + 1) * CHUNK].rearrange(
                        "p (s d) -> p s d", s=CHUNK_S
                    ),
                )
                nc.gpsimd.tensor_add(
                    out_sb[:, 0, ssl, :], out_sb[:, 1, ssl, :], se_sb[:]
                )
                if j_g == CHUNKS_PER_TILE - 1:
                    nc.gpsimd.dma_start(ov[t_g], out_sb[:])
                    del out_sbs[t_g]
                    del eps_tiles[t_g]

    nc.finalize()
    return nc


_NC_CACHE = None


def _get_nc():
    global _NC_CACHE
    if _NC_CACHE is None:
        _NC_CACHE = build_bass()
    return _NC_CACHE


def _run(inputs, trace=False, **kw):
    nc = _get_nc()
    xs = np.ascontiguousarray(np.asarray(inputs["x"])).astype(NPBF16)
    es = np.ascontiguousarray(np.asarray(inputs["eps"])).astype(NPBF16)
    weights = {
        k: np.ascontiguousarray(np.asarray(inputs[k], dtype=np.float32))
        for k in ("W_emb", "b_emb", "W_mean", "b_mean", "W_logvar", "b_logvar")
    }
    in_maps = []
    for c in range(N_CORES):
        sl = slice(c * ROWS_PER_CORE, (c + 1) * ROWS_PER_CORE)
        in_maps.append({"x": xs[sl], "eps": es[sl], **weights})
    res = run_bass_kernel_spmd(nc, in_maps, list(range(N_CORES)), trace=trace, **kw)
    z = np.concatenate(
        [res.results[c]["out"][0] for c in range(N_CORES)], axis=0
    ).astype(np.float32)
    mean = np.concatenate(
        [res.results[c]["out"][1] for c in range(N_CORES)], axis=0
    ).astype(np.float32)
    lv = np.concatenate(
        [res.results[c]["out"][2] for c in range(N_CORES)], axis=0
    ).astype(np.float32)
    return (z, mean, lv), res


def kernel(**inputs):
    out, _ = _run(inputs, trace=False)
    return out


if __name__ == "__main__":
    rng = np.random.default_rng(0)
    demo = {
        "x": rng.standard_normal((B, D_IN), dtype=np.float32),
        "eps": rng.standard_normal((B, D_OUT), dtype=np.float32),
        "W_emb": rng.standard_normal((D_IN, D_H), dtype=np.float32) * 0.088,
        "b_emb": rng.standard_normal((D_H,), dtype=np.float32) * 0.05,
        "W_mean": rng.standard_normal((D_H, D_OUT), dtype=np.float32) * 0.06,
        "b_mean": rng.standard_normal((D_OUT,), dtype=np.float32) * 0.03,
        "W_logvar": rng.standard_normal((D_H, D_OUT), dtype=np.float32) * 0.06,
        "b_logvar": rng.standard_normal((D_OUT,), dtype=np.float32) * 0.03,
    }
    z, m, l = kernel(**demo)
    print("shapes", z.shape, m.shape, l.shape)


# revision 22
# speedup vs baseline: 1.2048x; 1.1535x over previous
"""GaussianMLP sampling kernel for 8 trn2 NeuronCores (pure data parallel).

reference:
    h      = relu(x @ W_emb + b_emb)        x:[B,128] W_emb:[128,256]
    mean   = h @ W_mean + b_mean            W_mean:[256,128]
    logvar = h @ W_logvar + b_logvar        W_logvar:[256,128]
    z      = mean + exp(0.5*logvar) * eps
    returns (z, mean, logvar)

Sharding: x/eps split along batch across 8 cores; weights replicated.

v3 design (memory-regime):
  - All bulk I/O in bf16 (host converts): halves HBM traffic. Outputs are
    packed into one [3, R, 128] DRAM tensor, written with ONE DMA per
    2048-row tile (4 KiB contiguous runs per partition).
  - DRAM views "(t p s) d -> t p (s d)" keep per-partition runs >= 4 KiB.
  - PE per 512-row chunk: 4 bf16 transposes (512 cyc) + L1 (1024 cyc) +
    L2 (2048 cyc, 8 matmuls of 256 cols into a combined [mean|logvar]
    PSUM tile). No bias matmuls: L1 bias rides the ACT relu; L2 biases
    are added by DVE/Pool from precomputed broadcast tiles.
  - 5-stage software pipeline so every engine runs dependency-free:
      A: transpose(c) [PE] + PSUM->SBUF copy [DVE]
      B: L1(c-1) [PE] + relu0/1(c-1) [ACT]
      C: L2(c-2) [PE]
      D: +b_mean(c-3) [DVE], +b_logvar(c-3) [Pool]
      E: exp(c-4) [ACT], se=std*eps(c-4) [DVE], z=mean+se(c-4) [DVE],
         output DMA (per tile) [Pool SWDGE queue]
"""

import sys

sys.path.insert(0, "/opt/trn_rl_repo")

import numpy as np
import ml_dtypes

from contextlib import ExitStack

from concourse import bacc, bass, masks, mybir, tile
from concourse.alu_op_type import AluOpType
from concourse.bass_utils import run_bass_kernel_spmd

N_CORES = 8
B = 524288
D_IN = 128
D_H = 256
D_OUT = 128
ROWS_PER_CORE = B // N_CORES  # 65536

S_DMA = 16  # rows-per-partition per input DMA tile (2048 rows)
CHUNK_S = 4  # 512-row compute chunk = 4 x 128-row subtiles
CHUNK = CHUNK_S * 128
CHUNKS_PER_TILE = S_DMA // CHUNK_S  # 4
TILE_ROWS = 128 * S_DMA  # 2048

F32 = mybir.dt.float32
BF16 = mybir.dt.bfloat16
NPBF16 = ml_dtypes.bfloat16


def build_bass(rows_per_core=ROWS_PER_CORE):
    nc = bacc.Bacc("TRN2", target_bir_lowering=False, debug=False)
    n_tiles = rows_per_core // TILE_ROWS
    n_chunks = rows_per_core // CHUNK

    x_ext = nc.declare_dram_parameter("x", [rows_per_core, D_IN], BF16, isOutput=False)
    eps_ext = nc.declare_dram_parameter(
        "eps", [rows_per_core, D_OUT], BF16, isOutput=False
    )
    We_ext = nc.declare_dram_parameter("W_emb", [D_IN, D_H], F32, isOutput=False)
    be_ext = nc.declare_dram_parameter("b_emb", [D_H], F32, isOutput=False)
    Wm_ext = nc.declare_dram_parameter("W_mean", [D_H, D_OUT], F32, isOutput=False)
    bm_ext = nc.declare_dram_parameter("b_mean", [D_OUT], F32, isOutput=False)
    Wl_ext = nc.declare_dram_parameter("W_logvar", [D_H, D_OUT], F32, isOutput=False)
    bl_ext = nc.declare_dram_parameter("b_logvar", [D_OUT], F32, isOutput=False)
    out_ext = nc.declare_dram_parameter(
        "out", [3, rows_per_core, D_OUT], BF16, isOutput=True
    )

    # row = t*TILE_ROWS + p*S_DMA + s ; per-partition contiguous run = s*d
    xv = x_ext.rearrange("(t p s) d -> t p (s d)", p=128, s=S_DMA)
    ev = eps_ext.rearrange("(t p s) d -> t p (s d)", p=128, s=S_DMA)
    ov = out_ext.rearrange("c (t p s) d -> t p c s d", p=128, s=S_DMA)

    with tile.TileContext(nc) as tc, ExitStack() as ctx:
        const = ctx.enter_context(tc.tile_pool(name="const", bufs=1))
        xin = ctx.enter_context(tc.tile_pool(name="xin", bufs=3))
        epool = ctx.enter_context(tc.tile_pool(name="eps", bufs=4))
        xTp = ctx.enter_context(tc.tile_pool(name="xT", bufs=3))
        hTp = ctx.enter_context(tc.tile_pool(name="hTs", bufs=3))
        stdp = ctx.enter_context(tc.tile_pool(name="std", bufs=2))
        sep = ctx.enter_context(tc.tile_pool(name="se", bufs=2))
        outs = ctx.enter_context(tc.tile_pool(name="outs", bufs=2))
        psA = ctx.enter_context(tc.tile_pool(name="psA", bufs=2, space="PSUM"))
        psB0 = ctx.enter_context(tc.tile_pool(name="psB0", bufs=1, space="PSUM"))
        psB1 = ctx.enter_context(tc.tile_pool(name="psB1", bufs=1, space="PSUM"))
        psC = ctx.enter_context(tc.tile_pool(name="psC", bufs=2, space="PSUM"))

        # --- constants / weights (loaded once) ---
        ident = const.tile([128, 128], BF16)
        masks.make_identity(nc, ident[:])

        We_sb = const.tile([128, D_H], BF16)
        nc.gpsimd.dma_start(We_sb[:], We_ext[:])
        # combined [W_mean | W_logvar]: [k-chunk partition, k, 2*D_OUT]
        Wml_sb = const.tile([128, 2, 2 * D_OUT], BF16)
        nc.gpsimd.dma_start(
            Wml_sb[:, :, 0:D_OUT], Wm_ext.rearrange("(c p) d -> p c d", p=128)
        )
        nc.gpsimd.dma_start(
            Wml_sb[:, :, D_OUT : 2 * D_OUT],
            Wl_ext.rearrange("(c p) d -> p c d", p=128),
        )

        be_sb = const.tile([128, 2], F32)
        nc.sync.dma_start(be_sb[:], be_ext.rearrange("(c p) -> p c", p=128))

        # broadcast b_mean/b_logvar across partitions via one-time K=1
        # matmuls: [128,CHUNK] = ones[1,128].T @ bias_rep[1,CHUNK]
        ones_sb = const.tile([1, 128], F32)
        nc.vector.memset(ones_sb[:], 1.0)
        bm_rep = const.tile([1, CHUNK], F32)
        bl_rep = const.tile([1, CHUNK], F32)
        for s in range(CHUNK_S):
            nc.sync.dma_start(
                bm_rep[0:1, s * D_OUT : (s + 1) * D_OUT],
                bm_ext.rearrange("(o d) -> o d", o=1),
            )
            nc.sync.dma_start(
                bl_rep[0:1, s * D_OUT : (s + 1) * D_OUT],
                bl_ext.rearrange("(o d) -> o d", o=1),
            )
        bm_bc = const.tile([128, CHUNK_S, D_OUT], F32)
        bl_bc = const.tile([128, CHUNK_S, D_OUT], F32)
        binit_ps = psC.tile([128, CHUNK_S, 2 * D_OUT], F32, tag="ml")
        nc.tensor.matmul(
            binit_ps[:].rearrange("p s d -> p (s d)")[:, 0:CHUNK],
            ones_sb[:],
            bm_rep[:],
            start=True,
            stop=True,
            skip_group_check=True,
        )
        nc.vector.tensor_copy(
            bm_bc[:].rearrange("p s d -> p (s d)"),
            binit_ps[:].rearrange("p s d -> p (s d)")[:, 0:CHUNK],
        )
        binit2_ps = psC.tile([128, CHUNK_S, 2 * D_OUT], F32, tag="ml")
        nc.tensor.matmul(
            binit2_ps[:].rearrange("p s d -> p (s d)")[:, 0:CHUNK],
            ones_sb[:],
            bl_rep[:],
            start=True,
            stop=True,
            skip_group_check=True,
        )
        nc.vector.tensor_copy(
            bl_bc[:].rearrange("p s d -> p (s d)"),
            binit2_ps[:].rearrange("p s d -> p (s d)")[:, 0:CHUNK],
        )

        # --- pipelined main loop ---
        x_tiles = {}
        eps_tiles = {}
        xT_sbs = {}
        hT_sbs = {}
        ml_pss = {}
        out_sbs = {}
        std_sbs = {}

        def fetch_x(t):
            if t >= n_tiles:
                return
            x_sb = xin.tile([128, S_DMA * D_IN], BF16, tag="x")
            nc.sync.dma_start(x_sb[:], xv[t])
            x_tiles[t] = x_sb

        def fetch_eps(t):
            if t >= n_tiles:
                return
            e_sb = epool.tile([128, S_DMA * D_OUT], BF16, tag="eps")
            nc.sync.dma_start(e_sb[:], ev[t])
            eps_tiles[t] = e_sb

        for t0 in (0, 1):
            fetch_x(t0)
            fetch_eps(t0)

        for c in range(n_chunks + 5):
            # ---- stage E part 1: exp for chunk c-4 (inputs ready since
            # last iteration -> keeps ACT dense from the iteration start) ----
            g = c - 4
            if 0 <= g < n_chunks:
                t_g, j_g = divmod(g, CHUNKS_PER_TILE)
                _, m_t_g, l_t_g = out_sbs[t_g]
                ssl_g = slice(j_g * CHUNK_S, (j_g + 1) * CHUNK_S)
                std_sb = stdp.tile([128, CHUNK_S, D_OUT], BF16, tag="std")
                nc.scalar.activation(
                    std_sb[:],
                    l_t_g[:, ssl_g, :],
                    mybir.ActivationFunctionType.Exp,
                    scale=0.5,
                )
                std_sbs[g] = std_sb

            # ---- stage D: bias adds for chunk c-3 (L2 done last iter,
            # so these are ready first on DVE) ----
            f = c - 3
            if 0 <= f < n_chunks:
                t_f, j_f = divmod(f, CHUNKS_PER_TILE)
                if j_f == 0:
                    out_sbs[t_f] = (
                        outs.tile([128, S_DMA, D_OUT], BF16, tag="oz", name="z_t"),
                        outs.tile([128, S_DMA, D_OUT], BF16, tag="om", name="m_t"),
                        outs.tile([128, S_DMA, D_OUT], BF16, tag="ol", name="l_t"),
                    )
                z_t, m_t, l_t = out_sbs[t_f]
                ml_ps = ml_pss.pop(f)
                ssl = slice(j_f * CHUNK_S, (j_f + 1) * CHUNK_S)
                nc.vector.tensor_add(
                    m_t[:, ssl, :], ml_ps[:, :, 0:D_OUT], bm_bc[:]
                )
                nc.vector.tensor_add(
                    l_t[:, ssl, :], ml_ps[:, :, D_OUT : 2 * D_OUT], bl_bc[:]
                )
                if j_f == CHUNKS_PER_TILE - 1:
                    nc.gpsimd.dma_start(ov[t_f][:, 1, :, :], m_t[:])
                    nc.gpsimd.dma_start(ov[t_f][:, 2, :, :], l_t[:])

            # ---- stage A: transpose chunk c ----
            if c < n_chunks:
                t, j = divmod(c, CHUNKS_PER_TILE)
                if j == 0:
                    fetch_x(t + 2)
                elif j == 2:
                    fetch_eps(t + 2)
                x_sb = x_tiles[t]
                xT_ps = psA.tile([128, CHUNK], BF16, tag="xT")
                for q in range(CHUNK_S):
                    s = j * CHUNK_S + q
                    nc.tensor.transpose(
                        xT_ps[:, q * 128 : (q + 1) * 128],
                        x_sb[:, s * D_IN : (s + 1) * D_IN],
                        ident[:],
                    )
                xT_sb = xTp.tile([128, CHUNK], BF16, tag="xTs")
                nc.vector.tensor_copy(xT_sb[:], xT_ps[:])
                xT_sbs[c] = xT_sb
                if j == CHUNKS_PER_TILE - 1:
                    del x_tiles[t]

            # ---- stage B: L1 + relu for chunk c-1 ----
            d = c - 1
            if 0 <= d < n_chunks:
                xT_sb = xT_sbs.pop(d)
                hT_ps0 = psB0.tile([128, CHUNK], F32, tag="hT0")
                hT_ps1 = psB1.tile([128, CHUNK], F32, tag="hT1")
                nc.tensor.matmul(
                    hT_ps0[:], We_sb[:, 0:128], xT_sb[:], start=True, stop=True
                )
                nc.tensor.matmul(
                    hT_ps1[:], We_sb[:, 128:256], xT_sb[:], start=True, stop=True
                )
                hT_sb0 = hTp.tile([128, CHUNK], BF16, tag="h0")
                hT_sb1 = hTp.tile([128, CHUNK], BF16, tag="h1")
                nc.scalar.activation(
                    hT_sb0[:],
                    hT_ps0[:],
                    mybir.ActivationFunctionType.Relu,
                    bias=be_sb[:, 0:1],
                )
                nc.scalar.activation(
                    hT_sb1[:],
                    hT_ps1[:],
                    mybir.ActivationFunctionType.Relu,
                    bias=be_sb[:, 1:2],
                )
                hT_sbs[d] = (hT_sb0, hT_sb1)

            # ---- stage C: L2 for chunk c-2 ----
            e = c - 2
            if 0 <= e < n_chunks:
                hT_sb0, hT_sb1 = hT_sbs.pop(e)
                ml_ps = psC.tile([128, CHUNK_S, 2 * D_OUT], F32, tag="ml")
                for s in range(CHUNK_S):
                    sl = slice(s * 128, (s + 1) * 128)
                    for k, hT_sbk in ((0, hT_sb0), (1, hT_sb1)):
                        nc.tensor.matmul(
                            ml_ps[:, s, :],
                            hT_sbk[:, sl],
                            Wml_sb[:, k, :],
                            start=(k == 0),
                            stop=(k == 1),
                        )
                ml_pss[e] = ml_ps

            # ---- stage E part 2: sample / store for chunk c-4 ----
            if 0 <= g < n_chunks:
                t_g, j_g = divmod(g, CHUNKS_PER_TILE)
                z_t, m_t, l_t = out_sbs[t_g]
                ssl = slice(j_g * CHUNK_S, (j_g + 1) * CHUNK_S)
                std_sb = std_sbs.pop(g)
                e_sb = eps_tiles[t_g]
                se_sb = sep.tile([128, CHUNK_S, D_OUT], BF16, tag="se")
                nc.vector.tensor_mul(
                    se_sb[:],
                    std_sb[:],
                    e_sb[:, j_g * CHUNK : (j_g + 1) * CHUNK].rearrange(
                        "p (s d) -> p s d", s=CHUNK_S
                    ),
                )
                nc.gpsimd.tensor_add(z_t[:, ssl, :], m_t[:, ssl, :], se_sb[:])
                if j_g == CHUNKS_PER_TILE - 1:
                    nc.gpsimd.dma_start(ov[t_g][:, 0, :, :], z_t[:])
                    del out_sbs[t_g]
                    del eps_tiles[t_g]

    nc.finalize()
    return nc


_NC_CACHE = None


def _get_nc():
    global _NC_CACHE
    if _NC_CACHE is None:
        _NC_CACHE = build_bass()
    return _NC_CACHE


def _run(inputs, trace=False, **kw):
    nc = _get_nc()
    xs = np.ascontiguousarray(np.asarray(inputs["x"])).astype(NPBF16)
    es = np.ascontiguousarray(np.asarray(inputs["eps"])).astype(NPBF16)
    weights = {
        k: np.ascontiguousarray(np.asarray(inputs[k], dtype=np.float32))
        for k in ("W_emb", "b_emb", "W_mean", "b_mean", "W_logvar", "b_logvar")
    }
    in_maps = []
    for c in range(N_CORES):
        sl = slice(c * ROWS_PER_CORE, (c + 1) * ROWS_PER_CORE)
        in_maps.append({"x": xs[sl], "eps": es[sl], **weights})
    res = run_bass_kernel_spmd(nc, in_maps, list(range(N_CORES)), trace=trace, **kw)
    z = np.concatenate(
        [res.results[c]["out"][0] for c in range(N_CORES)], axis=0
    ).astype(np.float32)
    mean = np.concatenate(
        [res.results[c]["out"][1] for c in range(N_CORES)], axis=0
    ).astype(np.float32)
    lv = np.concatenate(
        [res.results[c]["out"][2] for c in range(N_CORES)], axis=0
    ).astype(np.float32)
    return (z, mean, lv), res


def kernel(**inputs):
    out, _ = _run(inputs, trace=False)
    return out


if __name__ == "__main__":
    rng = np.random.default_rng(0)
    demo = {
        "x": rng.standard_normal((B, D_IN), dtype=np.float32),
        "eps": rng.standard_normal((B, D_OUT), dtype=np.float32),
        "W_emb": rng.standard_normal((D_IN, D_H), dtype=np.float32) * 0.088,
        "b_emb": rng.standard_normal((D_H,), dtype=np.float32) * 0.05,
        "W_mean": rng.standard_normal((D_H, D_OUT), dtype=np.float32) * 0.06,
        "b_mean": rng.standard_normal((D_OUT,), dtype=np.float32) * 0.03,
        "W_logvar": rng.standard_normal((D_H, D_OUT), dtype=np.float32) * 0.06,
        "b_logvar": rng.standard_normal((D_OUT,), dtype=np.float32) * 0.03,
    }
    z, m, l = kernel(**demo)
    print("shapes", z.shape, m.shape, l.shape)
